# revision 8
# baseline (speedup 1.0000x reference)
"""Trainium2 Bass kernel for nn_CosineSimilarityLayer.

out = l2norm_rows(x) @ l2norm_rows_over_N(W)       x:[4096,512]  W:[512,5994]

Math:  out[b,n] = xscale[b] * sum_d x[b,d] * wscale[d] * W[d,n]
  xscale[b] = rsqrt(max(sum_d x[b,d]^2, eps))   (folded into PSUM eviction)
  wscale[d] = rsqrt(max(sum_n W[d,n]^2, eps))   (folded into transposed x)

Sharding: data-parallel over batch — 8 cores x [512, 512] x-shards, W
replicated.  Everything is core-local (no collectives): a cross-core sync
point bills every core for the slowest core's NEFF start (tens of us of
input-upload skew), which costs more than the redundant W-norm work saved.

Per-core pipeline:
  phase 1 (~40us, DMA-bound): x lands first, is squared (ACT) and
    PE-transposed via an identity; W streams in as 512-col chunks; each chunk
    is row-sum-squared on ACT and rounded to a resident float32r copy on DVE
    as it lands, so wscale is ready right after the last chunk.
  phase 2 (~45us, PE-bound): transposed x is scaled by wscale and rounded to
    float32r; then for each output tile 4 accumulating float32r matmuls run
    with the stationary operand shared across 4 chunk-PSUM banks; PSUM is
    evicted through ACT with the per-row xscale and stored via the
    Activation HWDGE ring (the Sync ring owns the W-in stream).

float32r is the fast fp32 matmul mode (~13-bit effective mantissa, full
speed for free dim >= 256).  NPASS=1 (default): rel err ~1.6e-4.  NPASS=3
splits each operand into value + residual (hi*hi + lo*hi + hi*lo) and
recovers ~fp32 accuracy (~2e-6) at 3x the PE time.
"""

import os
import sys
import types
from contextlib import ExitStack

import numpy as np


def _ensure_axon_hooks():
    """bass_utils' trace path imports antenv.axon_hooks, which some images
    lack.  Provide it (wired to the ctypes NTFF hook when available) so
    BASS_TRACE=1 profiles instead of crashing.  No-op when already present."""
    try:
        import antenv.axon_hooks  # noqa: F401
        return
    except ImportError:
        pass
    try:
        import antenv
    except ImportError:
        return
    m = types.ModuleType("antenv.axon_hooks")
    holder = {"h": None}
    m.set_axon_ntff_profile_hook = lambda h: holder.__setitem__("h", h)
    m.get_axon_ntff_profile_hook = lambda: holder["h"]
    sys.modules["antenv.axon_hooks"] = m
    antenv.axon_hooks = m
    try:
        from trn_agent_boot.trn_boot import _ntff_profile_via_ctypes
        so = "/opt/axon/libaxon_pjrt.so"
        if os.path.exists(so):
            m.set_axon_ntff_profile_hook(_ntff_profile_via_ctypes(so))
    except Exception:
        pass


_ensure_axon_hooks()

import concourse.bass as bass
import concourse.tile as tile
from concourse import bacc, mybir
from concourse.bass_utils import run_bass_kernel_spmd
from concourse.masks import make_identity

F32 = mybir.dt.float32
F32R = mybir.dt.float32r
AF = mybir.ActivationFunctionType
ALU = mybir.AluOpType

B, D, N = 4096, 512, 5994
NCORES = 8
P = 128
BSH = B // NCORES          # 512 rows of x per core
BT = BSH // P              # 4 b-tiles
DT = D // P                # 4 d-tiles (contraction)
CHUNK = 512                # output n-chunk (one PSUM bank of fp32)
GRP = 4                    # chunks per PSUM group in the matmul loop
EPS = 1e-12

NPASS = int(os.environ.get("COSSIM_NPASS", "1"))

CHUNKS = []
_n0 = 0
while _n0 < N:
    CHUNKS.append((_n0, min(CHUNK, N - _n0)))
    _n0 += CHUNK
NCH = len(CHUNKS)          # 12


def _x_prep(nc, tc, pools, x_r):
    """x load + row sumsq -> xscale; PE-transpose into xtf (fp32)."""
    const, xp, sq, sc, xt, tp = pools
    x_sb = xp.tile([P, BT, D], F32)
    nc.sync.dma_start(x_sb, x_r)
    xsq = sc.tile([P, BT], F32)
    for bt in range(BT):
        trash = sq.tile([P, D], F32, tag="trx")
        nc.scalar.activation(trash, x_sb[:, bt, :], AF.Square,
                             accum_out=xsq[:, bt:bt + 1])
    xmx = sc.tile([P, BT], F32)
    nc.vector.tensor_scalar_max(xmx, xsq, EPS)
    xsr = sc.tile([P, BT], F32)
    nc.scalar.sqrt(xsr, xmx)
    xsc = sc.tile([P, BT], F32)
    nc.vector.reciprocal(xsc, xsr)

    identity = const.tile([P, P], F32)
    make_identity(nc, identity)

    xtf = xt.tile([P, DT, BSH], F32, tag="xtf")
    for dt in range(DT):
        for bt in range(BT):
            pt = tp.tile([P, P], F32)
            nc.tensor.transpose(pt, x_sb[:, bt, dt * P:(dt + 1) * P], identity)
            nc.vector.tensor_copy(xtf[:, dt, bt * P:(bt + 1) * P], pt)
    return xsc, xtf


def _wscale_chain(nc, sc, wsqp):
    """Reduce per-chunk partials -> wscale = 1/sqrt(max(sum, eps))."""
    wsq = sc.tile([P, DT, 1], F32)
    nc.vector.reduce_sum(wsq, wsqp, axis=mybir.AxisListType.X)
    wmx = sc.tile([P, DT, 1], F32)
    nc.vector.tensor_scalar_max(wmx, wsq, EPS)
    wsr = sc.tile([P, DT, 1], F32)
    nc.scalar.sqrt(wsr, wmx)
    wsc = sc.tile([P, DT, 1], F32)
    nc.vector.reciprocal(wsc, wsr)
    return wsc


def _build_fast(nc, tc, ctx, x_r, w_r, o_r):
    """NPASS=1 path: W kept resident as rounded float32r."""
    const = ctx.enter_context(tc.tile_pool(name="const", bufs=1))
    xp = ctx.enter_context(tc.tile_pool(name="xp", bufs=1))
    sq = ctx.enter_context(tc.tile_pool(name="sq", bufs=2))
    sc = ctx.enter_context(tc.tile_pool(name="sc", bufs=1))
    xt = ctx.enter_context(tc.tile_pool(name="xt", bufs=1))
    wfp = ctx.enter_context(tc.tile_pool(name="wfp", bufs=3))
    wrs = ctx.enter_context(tc.tile_pool(name="wrs", bufs=1))
    ostp = ctx.enter_context(tc.tile_pool(name="ostp", bufs=3))
    tp = ctx.enter_context(tc.tile_pool(name="tp", bufs=2, space="PSUM"))
    mm = ctx.enter_context(tc.tile_pool(name="mm", bufs=4, space="PSUM"))

    xsc, xtf = _x_prep(nc, tc, (const, xp, sq, sc, xt, tp), x_r)

    # W stream: per chunk, ACT squares+row-accumulates (f32) while DVE
    # rounds into the resident f32r copy used by the matmuls.
    wr1 = wrs.tile([P, DT, N], F32R)
    wsqp = sc.tile([P, DT, NCH], F32)
    for ci, (n0, nw) in enumerate(CHUNKS):
        wf = wfp.tile([P, DT, CHUNK], F32, tag="wf")
        nc.sync.dma_start(wf[:, :, :nw], w_r[:, :, n0:n0 + nw])
        for dt in range(DT):
            trashw = sq.tile([P, CHUNK], F32, tag="trw")
            nc.scalar.activation(trashw[:, :nw], wf[:, dt, :nw], AF.Square,
                                 accum_out=wsqp[:, dt, ci:ci + 1])
            nc.vector.tensor_copy(wr1[:, dt, n0:n0 + nw], wf[:, dt, :nw])

    wsc = _wscale_chain(nc, sc, wsqp)

    xtr1 = xt.tile([P, DT, BSH], F32R, tag="xtr1")
    for dt in range(DT):
        nc.scalar.activation(xtr1[:, dt, :], xtf[:, dt, :], AF.Copy,
                             scale=wsc[:, dt, :])

    # Matmul loop: stationary operand xtr1[dt, bt] shared across a group of
    # GRP chunk-PSUMs; each PSUM accumulates its 4 K-tiles (start/stop).
    for bt in range(BT):
        for g0 in range(0, NCH, GRP):
            grp = CHUNKS[g0:g0 + GRP]
            gn0 = grp[0][0]
            gw = grp[-1][0] + grp[-1][1] - gn0
            pss = []
            for c in range(len(grp)):
                ps = mm.tile([P, CHUNK], F32, tag="ps")
                pss.append(ps)
            for dt in range(DT):
                for c, (n0, nw) in enumerate(grp):
                    nc.tensor.matmul(
                        pss[c][:, :nw],
                        xtr1[:, dt, bt * P:(bt + 1) * P],
                        wr1[:, dt, n0:n0 + nw],
                        start=(dt == 0), stop=(dt == DT - 1))
            ost = ostp.tile([P, GRP * CHUNK], F32, tag="ost")
            for c, (n0, nw) in enumerate(grp):
                nc.scalar.activation(ost[:, n0 - gn0:n0 - gn0 + nw],
                                     pss[c][:, :nw], AF.Copy,
                                     scale=xsc[:, bt:bt + 1])
            # Output DMA on the Activation HWDGE ring; the Sync ring owns
            # the W-in stream.
            nc.scalar.dma_start(o_r[:, bt, gn0:gn0 + gw], ost[:, :gw])


def _build_general(nc, tc, ctx, x_r, w_r, o_r, npass):
    """NPASS>=2 path: W resident in f32, per-chunk residual splitting."""
    const = ctx.enter_context(tc.tile_pool(name="const", bufs=1))
    xp = ctx.enter_context(tc.tile_pool(name="xp", bufs=1))
    sq = ctx.enter_context(tc.tile_pool(name="sq", bufs=2))
    sc = ctx.enter_context(tc.tile_pool(name="sc", bufs=1))
    xt = ctx.enter_context(tc.tile_pool(name="xt", bufs=1))
    wsb = ctx.enter_context(tc.tile_pool(name="wsb", bufs=1))
    wrp = ctx.enter_context(tc.tile_pool(name="wrp", bufs=3))
    ostp = ctx.enter_context(tc.tile_pool(name="ostp", bufs=3))
    tp = ctx.enter_context(tc.tile_pool(name="tp", bufs=2, space="PSUM"))
    mm = ctx.enter_context(tc.tile_pool(name="mm", bufs=4, space="PSUM"))

    xsc, xtf = _x_prep(nc, tc, (const, xp, sq, sc, xt, tp), x_r)

    w_sb = wsb.tile([P, DT, N], F32)
    wsqp = sc.tile([P, DT, NCH], F32)
    for ci, (n0, nw) in enumerate(CHUNKS):
        nc.sync.dma_start(w_sb[:, :, n0:n0 + nw], w_r[:, :, n0:n0 + nw])
        for dt in range(DT):
            trashw = sq.tile([P, CHUNK], F32, tag="trw")
            nc.scalar.activation(trashw[:, :nw], w_sb[:, dt, n0:n0 + nw],
                                 AF.Square, accum_out=wsqp[:, dt, ci:ci + 1])

    wsc = _wscale_chain(nc, sc, wsqp)

    xtr1 = xt.tile([P, DT, BSH], F32R, tag="xtr1")
    xtr2 = xt.tile([P, DT, BSH], F32R, tag="xtr2")
    for dt in range(DT):
        nc.scalar.activation(xtr1[:, dt, :], xtf[:, dt, :], AF.Copy,
                             scale=wsc[:, dt, :])
        nc.vector.scalar_tensor_tensor(
            out=xtr2[:, dt, :], in0=xtf[:, dt, :], scalar=wsc[:, dt, :],
            in1=xtr1[:, dt, :], op0=ALU.mult, op1=ALU.subtract)

    for n0, nw in CHUNKS:
        wr1 = wrp.tile([P, DT, CHUNK], F32R, tag="wr1")
        for dt in range(DT):
            nc.vector.tensor_copy(wr1[:, dt, :nw], w_sb[:, dt, n0:n0 + nw])
        wr2 = None
        if npass >= 3:
            wr2 = wrp.tile([P, DT, CHUNK], F32R, tag="wr2")
            for dt in range(DT):
                nc.vector.scalar_tensor_tensor(
                    out=wr2[:, dt, :nw], in0=w_sb[:, dt, n0:n0 + nw],
                    scalar=1.0, in1=wr1[:, dt, :nw],
                    op0=ALU.mult, op1=ALU.subtract)

        terms = [(xtr1, wr1), (xtr2, wr1)]
        if npass >= 3:
            terms.append((xtr1, wr2))

        ost = ostp.tile([P, BT, CHUNK], F32, tag="ost")
        nmm = len(terms) * DT
        for bt in range(BT):
            ps = mm.tile([P, CHUNK], F32)
            i = 0
            for xs, ws in terms:
                for dt in range(DT):
                    nc.tensor.matmul(
                        ps[:, :nw],
                        xs[:, dt, bt * P:(bt + 1) * P],
                        ws[:, dt, :nw],
                        start=(i == 0), stop=(i == nmm - 1))
                    i += 1
            nc.scalar.activation(ost[:, bt, :nw], ps[:, :nw], AF.Copy,
                                 scale=xsc[:, bt:bt + 1])
        nc.scalar.dma_start(o_r[:, :, n0:n0 + nw], ost[:, :, :nw])


def _build(npass: int):
    nc = bacc.Bacc("TRN2", target_bir_lowering=False, debug=False,
                   num_devices=NCORES)

    x_d = nc.dram_tensor("x", [BSH, D], F32, kind="ExternalInput").ap()
    w_d = nc.dram_tensor("W", [D, N], F32, kind="ExternalInput").ap()
    o_d = nc.dram_tensor("out", [BSH, N], F32, kind="ExternalOutput").ap()

    x_r = x_d.rearrange("(t p) d -> p t d", p=P)        # [128, 4, 512]
    w_r = w_d.rearrange("(t p) n -> p t n", p=P)        # [128, 4, 5994]
    o_r = o_d.rearrange("(t p) n -> p t n", p=P)        # [128, 4, 5994]

    with tile.TileContext(nc) as tc, ExitStack() as ctx:
        if npass <= 1:
            _build_fast(nc, tc, ctx, x_r, w_r, o_r)
        else:
            _build_general(nc, tc, ctx, x_r, w_r, o_r, npass)

    nc.compile()
    return nc


LAST_RESULT = None


def kernel(x: np.ndarray, W: np.ndarray) -> np.ndarray:
    global LAST_RESULT
    x = np.ascontiguousarray(x, dtype=np.float32)
    W = np.ascontiguousarray(W, dtype=np.float32)
    assert x.shape == (B, D) and W.shape == (D, N)

    nc = _build(NPASS)

    in_maps = [{"x": np.ascontiguousarray(x[c * BSH:(c + 1) * BSH]), "W": W}
               for c in range(NCORES)]

    res = run_bass_kernel_spmd(nc, in_maps, core_ids=list(range(NCORES)))
    LAST_RESULT = res
    return np.concatenate([res.results[c]["out"] for c in range(NCORES)],
                          axis=0)
